# revision 47
# baseline (speedup 1.0000x reference)
"""NetGINE (4-layer GIN message passing) on 8 Trainium2 NeuronCores.

Sharding: nodes/edges sharded by destination across 8 cores (6400 padded node
slots per core). Node tables are bf16 pair-rows [NPAD/2, 128] so a 256B gather
element covers one (even,odd) node pair; edges are bucketed per dst tile by
src parity and gather h[src] via gpsimd dma_gather (4 SWDGE queues).

Per layer the work is software-pipelined: "prep" (bond encoder e and scatter
one-hots via tgen matmul + is_equal — independent of gathered data) runs two
chunks ahead of "consume" (gather + add + relu + one-hot scatter matmuls into
PSUM), so gather descriptor generation on the Q7s — the pacing resource at
~8ns/idx with 2 concurrent streams — never waits on compute. Node MLP + BN
run on transposed [64, nodes] tiles interleaved with the chunk stream; the
bf16 AllGather rebuilds the pair-row table between layers while per-layer
mean-pooling (one-hot matmuls) and the next layer's prep fill its shadow;
bf16 pooled partials are AllReduced once at the end, head MLP replicated.
"""

import os
import numpy as np
import ml_dtypes

BF16 = np.dtype(ml_dtypes.bfloat16)

N, E, G, DIM, XF, EF = 50000, 800000, 512, 64, 28, 3
NCORES = 8
NLOC = 6400              # padded node slots per core
NPAD = NCORES * NLOC     # 51200
TILES = NLOC // 128      # 50
NROWP = NPAD // 2        # 25600 pair-rows in the bf16 table
TPC = 2                  # tiles per chunk
NLAYERS = 4
GWIN = 4                 # 128-graph pooling windows


# ---------------------------------------------------------------- host prep --

def _plan_nodes(batch):
    """Assign nodes to per-core padded slots; no 128-slot tile may span a
    128-graph window boundary."""
    slot2node = np.full((NCORES, NLOC), -1, np.int64)
    node2pad = np.full(N, -1, np.int64)
    per_core = N // NCORES  # 6250
    for c in range(NCORES):
        nodes = np.arange(c * per_core, (c + 1) * per_core)
        wins = batch[nodes] // 128
        change = np.nonzero(np.diff(wins))[0] + 1
        bounds = [0] + list(change) + [len(nodes)]
        s = 0
        for i in range(len(bounds) - 1):
            lo, hi = bounds[i], bounds[i + 1]
            if i > 0 and s % 128 != 0:
                s += 128 - (s % 128)
            cnt = hi - lo
            assert s + cnt <= NLOC, "node padding overflow"
            slot2node[c, s:s + cnt] = nodes[lo:hi]
            node2pad[nodes[lo:hi]] = c * NLOC + s + np.arange(cnt)
            s += cnt
    return slot2node, node2pad


def _prep(inputs):
    x = np.asarray(inputs["x"], np.float32)
    edge_attr = np.asarray(inputs["edge_attr"], np.float32)
    edge_index = np.asarray(inputs["edge_index"], np.int64)
    batch = np.asarray(inputs["batch"], np.int64)

    slot2node, node2pad = _plan_nodes(batch)

    src_p = node2pad[edge_index[0]]
    dst_p = node2pad[edge_index[1]]
    core = dst_p // NLOC
    dslot = dst_p % NLOC
    tile_of = dslot // 128
    drel = dslot % 128
    grp_b = src_p % 2                    # 0 = even column, 1 = odd column
    idx_val = src_p // 2                 # pair-row index (< 25600, int16-safe)

    key = (core * TILES + tile_of) * 2 + grp_b
    counts = np.bincount(key, minlength=NCORES * TILES * 2).reshape(NCORES, TILES, 2)
    BA = max(int(np.ceil(counts[:, :, 0].max() / 128)), 1)
    BB = max(int(np.ceil(counts[:, :, 1].max() / 128)), 1)
    if (BA + BB) % 2 == 1:
        BB += 1
    NBT = BA + BB
    NBLK = TILES * NBT
    SLOTS = NBLK * 128

    chunk_tiles = [list(range(t, min(t + TPC, TILES)))
                   for t in range(0, TILES, TPC)]

    slot_base = {}
    s0 = 0
    for tl in chunk_tiles:
        for ti in tl:
            slot_base[(ti, 0)] = s0
            s0 += BA * 128
        for ti in tl:
            slot_base[(ti, 1)] = s0
            s0 += BB * 128
    assert s0 == SLOTS

    # slot of each edge: base of its (tile, group) + rank within that list
    order = np.argsort(key, kind="stable")
    ends = np.cumsum(counts.reshape(-1))
    starts = ends - counts.reshape(-1)
    rank = np.empty(E, np.int64)
    rank[order] = np.arange(E) - starts[key[order]]
    base_arr = np.zeros((NCORES, TILES, 2), np.int64)
    for ti in range(TILES):
        for g in range(2):
            base_arr[:, ti, g] = slot_base[(ti, g)]
    slot_of_edge = base_arr.reshape(-1)[key] + rank

    gidx = np.zeros((NCORES, 128, SLOTS // 16), np.int16)
    dstrel = np.full((NCORES, 128, NBLK), -1e9, np.float32)
    easl = np.zeros((NCORES, SLOTS, EF), np.float32)
    for c in range(NCORES):
        m = core == c
        sl = slot_of_edge[m]
        dstrel[c][sl % 128, sl // 128] = drel[m]
        easl[c][sl] = edge_attr[m]
        iv = np.zeros(SLOTS, np.int64)
        iv[sl] = idx_val[m]
        col0 = 0
        for tl in chunk_tiles:
            for g, BG in ((0, BA), (1, BB)):
                L = len(tl) * BG * 128
                s_lo = slot_base[(tl[0], g)]
                j = np.arange(L)
                gidx[c][j % 16, col0 + j // 16] = iv[s_lo:s_lo + L].astype(np.int16)
                col0 += L // 16
        gidx[c] = np.tile(gidx[c][:16], (8, 1))

    # dstrelT: chunk-local groups of 4 blocks for the one-hot tgen matmul
    ngs = []          # per-chunk group count
    for tl in chunk_tiles:
        ngs.append(-(-len(tl) * NBT // 4))
    NGRP = sum(ngs)
    dstrelT = np.full((NCORES, 8, NGRP, 128), -1e9, np.float32)
    for c in range(NCORES):
        gb = 0
        for ci, tl in enumerate(chunk_tiles):
            nb = len(tl) * NBT
            b0 = slot_base[(tl[0], 0)] // 128
            for g in range(ngs[ci]):
                for q in range(4):
                    b = 4 * g + q
                    if b < nb:
                        dstrelT[c, q, gb + g, :] = dstrel[c][:, b0 + b]
            gb += ngs[ci]
        dstrelT[c, 4] = 1.0
    tgen_rhs = np.zeros((8, 512), np.float32)
    for q in range(4):
        tgen_rhs[q, 128 * q:128 * (q + 1)] = 1.0
    tgen_rhs[4] = -np.tile(np.arange(128, dtype=np.float32), 4)

    # eaT2 stacked pairing: unit u covers slots [256u,256u+128) top, +128 bottom
    easl_u = easl.reshape(NCORES, SLOTS // 256, 2, 128, EF)
    eaT2 = np.zeros((NCORES, 2 * EF, SLOTS // 2), BF16)
    for c in range(NCORES):
        eaT2[c, :EF] = easl_u[c, :, 0].transpose(2, 0, 1).reshape(EF, -1).astype(BF16)
        eaT2[c, EF:] = easl_u[c, :, 1].transpose(2, 0, 1).reshape(EF, -1).astype(BF16)

    # node-side tensors: bf16 pair-row table + transposed f32 h
    xpad = np.zeros((N, DIM), np.float32)
    xpad[:, :XF] = x
    T1 = np.zeros((NPAD, DIM), np.float32)
    flat = slot2node.reshape(-1)
    valid = flat >= 0
    T1[valid] = xpad[flat[valid]]
    T1 = T1.astype(BF16).reshape(NROWP, 2 * DIM)
    hT0 = np.zeros((NCORES, DIM, NLOC), np.float32)
    gid4 = np.full((NCORES, 128, GWIN * TILES), -1e9, np.float32)
    for c in range(NCORES):
        sn = slot2node[c]
        v = sn >= 0
        hT0[c][:, v.nonzero()[0]] = xpad[sn[v]].T
        gid = np.full(NLOC, -1e9)
        gid[v] = batch[sn[v]]
        for w in range(GWIN):
            gid4[c][:, w * TILES:(w + 1) * TILES] = \
                (gid - 128 * w).reshape(TILES, 128).T

    cntg = np.bincount(batch, minlength=G).astype(np.float32)
    inv_cnt = (1.0 / np.maximum(cntg, 1.0)).reshape(GWIN, 128).T.copy()

    def padw(a, r, cc):
        out = np.zeros((r, cc), np.float32)
        a = np.asarray(a, np.float32)
        out[:a.shape[0], :a.shape[1]] = a
        return out

    wb = {}
    for li, p in ((1, "c1"), (2, "c2"), (3, "c3")):
        be1 = padw(inputs[f"{p}_be1"], EF, DIM)
        be2 = padw(inputs[f"{p}_be2"], DIM, DIM)
        be1_2 = np.zeros((2 * EF, 128), np.float32)
        be1_2[:EF, :DIM] = be1
        be1_2[EF:, DIM:] = be1
        be2_2 = np.zeros((128, 128), np.float32)
        be2_2[:DIM, :DIM] = be2
        be2_2[DIM:, DIM:] = be2
        wb[f"be1_{li}"] = be1_2.astype(BF16)
        wb[f"be2_{li}"] = be2_2.astype(BF16)
        wb[f"m1_{li}"] = padw(inputs[f"{p}_m1"], DIM, DIM)
        wb[f"m2_{li}"] = padw(inputs[f"{p}_m2"], DIM, DIM)
    eps = [float(np.asarray(inputs[f"{p}_eps"]).reshape(-1)[0])
           for p in ("c1", "c2", "c3")]
    epsv = np.array([[eps[0]], [eps[1]], [eps[2]], [eps[2]]], np.float32)

    common = {
        "t0": T1,
        "iota": np.tile(np.arange(128, dtype=np.float32), (128, 1)),
        "idf32": np.eye(128, dtype=np.float32),
        "idbf": np.eye(128, dtype=np.float32).astype(BF16),
        "epsv": epsv,
        "inv_cnt": inv_cnt,
        "fc1_w": np.asarray(inputs["fc1_w"], np.float32).reshape(2, 128, DIM)
                   .transpose(1, 0, 2).copy(),
        "fc1_b": np.asarray(inputs["fc1_b"], np.float32).reshape(DIM, 1),
        "fc2_w": np.asarray(inputs["fc2_w"], np.float32),
        "fc2_b": np.asarray(inputs["fc2_b"], np.float32).reshape(DIM, 1),
        "fc3_w": np.asarray(inputs["fc3_w"], np.float32),
        "fc3_b": np.asarray(inputs["fc3_b"], np.float32).reshape(DIM, 1),
        "fc4_w": np.asarray(inputs["fc4_w"], np.float32),
        "fc4_b": np.asarray(inputs["fc4_b"], np.float32).reshape(1, 1),
    }
    common.update(wb)
    for i in range(1, 5):
        for s in "gbmv":
            common[f"bn{i}_{s}"] = np.asarray(inputs[f"bn{i}_{s}"],
                                              np.float32).reshape(DIM, 1)

    common["tgen_rhs"] = tgen_rhs.astype(BF16)

    in_maps = []
    for c in range(NCORES):
        m = dict(common)
        m["gidx"] = gidx[c]
        m["dstrelT"] = dstrelT[c].astype(BF16)
        m["eaT2"] = eaT2[c]
        m["hT0"] = hT0[c].astype(BF16)
        m["gid4"] = gid4[c]
        in_maps.append(m)

    struct = dict(BA=BA, BB=BB, NBT=NBT, NBLK=NBLK, SLOTS=SLOTS, NGRP=NGRP,
                  ngs=ngs, chunk_tiles=chunk_tiles, slot_base=slot_base)
    return in_maps, struct


# ------------------------------------------------------------- bass program --

def _build(struct):
    from concourse import bacc, tile, mybir
    f32, bf16, i16 = mybir.dt.float32, mybir.dt.bfloat16, mybir.dt.int16
    Alu = mybir.AluOpType
    Act = mybir.ActivationFunctionType

    BA, BB, NBT = struct["BA"], struct["BB"], struct["NBT"]
    SLOTS = struct["SLOTS"]
    NGRP = struct["NGRP"]
    ngs = struct["ngs"]
    chunk_tiles = struct["chunk_tiles"]
    NB = TPC * NBT            # max blocks per chunk
    NGC = -(-NB // 4)         # max one-hot groups per chunk

    nc = bacc.Bacc("TRN2", target_bir_lowering=False, debug=False,
                   num_devices=NCORES, num_swdge_queues=4)

    def din(name, shape, dt=f32):
        return nc.dram_tensor(name, shape, dt, kind="ExternalInput")

    t0 = din("t0", [NROWP, 2 * DIM], bf16)
    gidx_d = din("gidx", [128, SLOTS // 16], i16)
    dstrelT_d = din("dstrelT", [8, NGRP, 128], bf16)
    tgen_d = din("tgen_rhs", [8, 512], bf16)
    eaT2_d = din("eaT2", [2 * EF, SLOTS // 2], bf16)
    hT0_d = din("hT0", [DIM, NLOC], bf16)
    gid4_d = din("gid4", [128, GWIN * TILES])
    iota_d = din("iota", [128, 128])
    idf32_d = din("idf32", [128, 128])
    idbf_d = din("idbf", [128, 128], bf16)
    epsv_d = din("epsv", [4, 1])
    invc_d = din("inv_cnt", [128, GWIN])
    wdict = {}
    for li in (1, 2, 3):
        wdict[f"be1_{li}"] = din(f"be1_{li}", [2 * EF, 128], bf16)
        wdict[f"be2_{li}"] = din(f"be2_{li}", [128, 128], bf16)
        wdict[f"m1_{li}"] = din(f"m1_{li}", [DIM, DIM])
        wdict[f"m2_{li}"] = din(f"m2_{li}", [DIM, DIM])
    for i in range(1, 5):
        for s in "gbmv":
            wdict[f"bn{i}_{s}"] = din(f"bn{i}_{s}", [DIM, 1])
    fc1_w = din("fc1_w", [128, 2, DIM])
    fc2_w = din("fc2_w", [DIM, DIM])
    fc3_w = din("fc3_w", [DIM, DIM])
    fc4_w = din("fc4_w", [DIM, 1])
    fcb_d = {"b1": din("fc1_b", [DIM, 1]), "b2": din("fc2_b", [DIM, 1]),
             "b3": din("fc3_b", [DIM, 1]), "b4": din("fc4_b", [1, 1])}

    out_d = nc.dram_tensor("out", [1, G], f32, kind="ExternalOutput")
    bounce = [nc.dram_tensor(f"bounce{l}", [NLOC, DIM], bf16)
              for l in range(NLAYERS - 1)]
    tables = [t0] + [nc.dram_tensor(f"T{l}", [NROWP, 2 * DIM], bf16,
                                    addr_space="Shared")
                     for l in (1, 2, 3)]
    poolohs_d = nc.dram_tensor("poolohs", [128, GWIN, TILES, 128], bf16)
    arin_d = nc.dram_tensor("arin", [128, GWIN, 4 * DIM], bf16)
    arout_d = nc.dram_tensor("arout", [128, GWIN, 4 * DIM], bf16,
                             addr_space="Shared")

    qctr = [0]

    def next_q():
        q = qctr[0] % 4
        qctr[0] += 1
        return q

    with tile.TileContext(nc) as tc:
        with tc.tile_pool(name="res", bufs=1) as res, \
             tc.tile_pool(name="hsrcp", bufs=3) as hsrcp, \
             tc.tile_pool(name="msgp", bufs=2) as msgp, \
             tc.tile_pool(name="e1p", bufs=2) as e1p, \
             tc.tile_pool(name="eap", bufs=2) as eap, \
             tc.tile_pool(name="ohp", bufs=3) as ohp, \
             tc.tile_pool(name="esbp", bufs=3) as esbp, \
             tc.tile_pool(name="pohp", bufs=2) as pohp, \
             tc.tile_pool(name="smallp", bufs=2) as smallp, \
             tc.tile_pool(name="psA", bufs=3, space="PSUM") as psA, \
             tc.tile_pool(name="psB", bufs=1, space="PSUM") as psB, \
             tc.tile_pool(name="psP", bufs=2, space="PSUM") as psP, \
             tc.tile_pool(name="psG", bufs=2, space="PSUM") as psG:

            # ---------------- residents
            def load(name, shape, dt, dram):
                tl_ = res.tile(shape, dt, tag=name)
                nc.sync.dma_start(out=tl_[:], in_=dram[:])
                return tl_

            iota_sb = load("iota", [128, 128], f32, iota_d)
            tgen_sb = load("tgen", [8, 512], bf16, tgen_d)
            idf_sb = load("idf", [128, 128], f32, idf32_d)
            idbf_sb = load("idbf", [128, 128], bf16, idbf_d)
            gid4_sb = load("gid4", [128, GWIN * TILES], f32, gid4_d)
            invc_sb = load("invc", [128, GWIN], f32, invc_d)
            gidx_sb = load("gidx", [128, SLOTS // 16], i16, gidx_d)
            w_sb = {k: load(f"w_{k}", list(d.shape), d.dtype, d)
                    for k, d in wdict.items()}
            fc1w_sb = load("fc1w", [128, 2, DIM], f32, fc1_w)
            fc2w_sb = load("fc2w", [DIM, DIM], f32, fc2_w)
            fc3w_sb = load("fc3w", [DIM, DIM], f32, fc3_w)
            fc4w_sb = load("fc4w", [DIM, 1], f32, fc4_w)
            fcb_sb = {k: load(f"fcb{k}", list(d.shape), f32, d)
                      for k, d in fcb_d.items()}
            hT = load("hT", [DIM, NLOC], bf16, hT0_d)
            aggT = res.tile([DIM, NLOC], bf16, tag="aggT")
            xn_l = res.tile([128, TILES, DIM], bf16, tag="xn_l")
            pooled = res.tile([128, GWIN, 4 * DIM], bf16, tag="pooled")

            # eps broadcast [64,1] per layer: (1+eps)
            eps1p = []
            for l in range(NLAYERS):
                e0 = res.tile([1, 1], f32, tag=f"eps0_{l}")
                nc.sync.dma_start(out=e0[:], in_=epsv_d[l:l + 1, :])
                eb = res.tile([DIM, 1], f32, tag=f"epsb{l}")
                nc.gpsimd.partition_broadcast(eb[:], e0[:], channels=DIM)
                e1 = res.tile([DIM, 1], f32, tag=f"eps1p{l}")
                nc.vector.tensor_scalar_add(e1[:], eb[:], 1.0)
                eps1p.append(e1)

            # bn params -> scale g', shift b'
            bn_s, bn_t = [], []
            for i in range(1, 5):
                v = w_sb[f"bn{i}_v"]; gg = w_sb[f"bn{i}_g"]
                bb = w_sb[f"bn{i}_b"]; mm = w_sb[f"bn{i}_m"]
                ve = res.tile([DIM, 1], f32, tag=f"bnve{i}")
                nc.vector.tensor_scalar_add(ve[:], v[:], 1e-5)
                sq = res.tile([DIM, 1], f32, tag=f"bnsq{i}")
                nc.scalar.activation(sq[:], ve[:], Act.Sqrt)
                inv = res.tile([DIM, 1], f32, tag=f"bninv{i}")
                nc.vector.reciprocal(inv[:], sq[:])
                gp = res.tile([DIM, 1], f32, tag=f"bngp{i}")
                nc.vector.tensor_mul(gp[:], gg[:], inv[:])
                tt = res.tile([DIM, 1], f32, tag=f"bntt{i}")
                nc.vector.tensor_mul(tt[:], mm[:], gp[:])
                bp = res.tile([DIM, 1], f32, tag=f"bnbp{i}")
                nc.vector.tensor_sub(bp[:], bb[:], tt[:])
                bn_s.append(gp); bn_t.append(bp)

            # ------- layers: software-pipelined prep (bond enc + one-hots,
            # no gather dependency) / consume (gather + add + relu + scatter)
            NREG = -(-NLOC // 512)
            nch = len(chunk_tiles)
            DP = 2                       # prep-ahead depth in chunks
            ng_base = []
            _a = 0
            for ci in range(nch):
                ng_base.append(_a)
                _a += ngs[ci]
            ohc_t = {}
            esb_t = {}

            def prep(l, ci):
                wl = min(l + 1, 3)
                be1 = w_sb[f"be1_{wl}"]; be2 = w_sb[f"be2_{wl}"]
                tl = chunk_tiles[ci]
                ntl = len(tl)
                nb = ntl * NBT
                ncols = nb * 64
                s0 = struct["slot_base"][(tl[0], 0)]
                ng = ngs[ci]
                # one-hots: t = dstrel - n via PE, is_eq(imm 0) on DVE
                dsl = smallp.tile([8, NGC, 128], bf16, tag="dsl", bufs=3)
                nc.sync.dma_start(out=dsl[:, 0:ng, :],
                                  in_=dstrelT_d[:, ng_base[ci]:ng_base[ci] + ng, :])
                ohc = ohp.tile([128, NGC, 512], bf16, tag="ohc")
                for gq in range(ng):
                    pst4 = psA.tile([128, 512], f32, tag="ps1", name="pst4")
                    nc.tensor.matmul(pst4[:], dsl[:, gq, :], tgen_sb[:],
                                     start=True, stop=True)
                    nc.vector.tensor_single_scalar(ohc[:, gq, :], pst4[:], 0.0,
                                                   Alu.is_equal)
                # bond encoder stage 1
                c0 = s0 // 2
                ea_sb = eap.tile([2 * EF, NB * 64], bf16, tag="ea")
                nc.sync.dma_start(out=ea_sb[:, 0:ncols],
                                  in_=eaT2_d[:, c0:c0 + ncols])
                e1t = e1p.tile([128, NB * 64], bf16, tag="e1")
                g0 = 0
                while g0 < ncols:
                    gw = min(512, ncols - g0)
                    ps1 = psA.tile([128, 512], f32, tag="ps1")
                    nc.tensor.matmul(ps1[:, 0:gw], be1[:], ea_sb[:, g0:g0 + gw],
                                     start=True, stop=True)
                    nc.scalar.activation(e1t[:, g0:g0 + gw], ps1[:, 0:gw],
                                         Act.Relu)
                    g0 += gw
                # stage 2 (pair matmul) -> e materialized bf16 in SBUF
                esb = esbp.tile([128, NB, DIM], bf16, tag="esb")
                for gb8 in range(0, nb, 8):
                    gwb = min(8, nb - gb8)
                    pse = psP.tile([128, 8, DIM], f32, tag="pse")
                    for j in range(gwb // 2):
                        u = gb8 // 2 + j
                        nc.tensor.matmul(pse[:, 2 * j:2 * j + 2, :],
                                         e1t[:, 128 * u:128 * (u + 1)],
                                         be2[:], start=True, stop=True)
                    nc.scalar.activation(esb[:, gb8:gb8 + gwb, :],
                                         pse[:, 0:gwb, :], Act.Identity)
                ohc_t[(l, ci)] = ohc
                esb_t[(l, ci)] = esb

            for l in range(NLAYERS):
                m1 = w_sb[f"m1_{min(l + 1, 3)}"]
                m2 = w_sb[f"m2_{min(l + 1, 3)}"]
                tbl = tables[l]
                state = {"r": 0, "t": 0}

                def node_region(r):
                    g0 = 512 * r
                    gw = min(512, NLOC - g0)
                    sl = slice(g0, g0 + gw)
                    zT = smallp.tile([DIM, 512], f32, tag="zT")
                    nc.vector.scalar_tensor_tensor(zT[:, 0:gw], hT[:, sl],
                                                   eps1p[l][:], aggT[:, sl],
                                                   Alu.mult, Alu.add)
                    ps1 = psA.tile([128, 512], f32, tag="ps1", name="mlp1")
                    nc.tensor.matmul(ps1[0:DIM, 0:gw], m1[:], zT[:, 0:gw],
                                     start=True, stop=True)
                    r1 = smallp.tile([DIM, 512], f32, tag="r1")
                    nc.scalar.activation(r1[:, 0:gw], ps1[0:DIM, 0:gw], Act.Relu)
                    ps2 = psB.tile([128, 512], f32, tag="ps2")
                    nc.tensor.matmul(ps2[0:DIM, 0:gw], m2[:], r1[:, 0:gw],
                                     start=True, stop=True)
                    rr = smallp.tile([DIM, 512], f32, tag="rr")
                    nc.scalar.activation(rr[:, 0:gw], ps2[0:DIM, 0:gw], Act.Relu)
                    nc.vector.tensor_scalar(hT[:, sl], rr[:, 0:gw], bn_s[l][:],
                                            bn_t[l][:], Alu.mult, Alu.add)

                def tile_post(ti):
                    pst = psP.tile([128, 16, DIM], bf16, tag="pse", name="ptr")
                    nc.tensor.transpose(pst[:, 0, :],
                                        hT[:, 128 * ti:128 * (ti + 1)],
                                        idbf_sb[0:DIM, 0:DIM])
                    nc.vector.tensor_copy(xn_l[:, ti, :], pst[:, 0, :])
                    if l < NLAYERS - 1:
                        nc.sync.dma_start(
                            out=bounce[l][128 * ti:128 * (ti + 1), :],
                            in_=xn_l[:, ti, :])

                def flush(tiles_done, final=False):
                    while state["r"] < NREG and \
                            (final or min(4 * (state["r"] + 1), TILES) <= tiles_done):
                        node_region(state["r"])
                        state["r"] += 1
                    tmax = min(4 * state["r"], TILES)
                    while state["t"] < tmax:
                        tile_post(state["t"])
                        state["t"] += 1

                if l == 0:
                    for j in range(min(DP, nch)):
                        prep(0, j)

                for ci, tl in enumerate(chunk_tiles):
                    ntl = len(tl)
                    nb = ntl * NBT
                    s0 = struct["slot_base"][(tl[0], 0)]
                    assert s0 % 256 == 0
                    la = ntl * BA * 128
                    lb = ntl * BB * 128
                    nea = ntl * BA          # even-parity block count
                    hsrc = hsrcp.tile([128, NB, 2 * DIM], bf16, tag="hsrc")
                    nc.gpsimd.dma_gather(
                        out_ap=hsrc[:, 0:nea, :], in_ap=tbl[:, :],
                        idxs_ap=gidx_sb[:, s0 // 16:(s0 + la) // 16],
                        num_idxs=la, num_idxs_reg=la, elem_size=2 * DIM,
                        single_packet=False, queue_num=next_q())
                    nc.gpsimd.dma_gather(
                        out_ap=hsrc[:, nea:nb, :], in_ap=tbl[:, :],
                        idxs_ap=gidx_sb[:, (s0 + la) // 16:(s0 + la + lb) // 16],
                        num_idxs=lb, num_idxs_reg=lb, elem_size=2 * DIM,
                        single_packet=False, queue_num=next_q())

                    ohc = ohc_t.pop((l, ci))
                    esb = esb_t.pop((l, ci))
                    msg = msgp.tile([128, NB, DIM], bf16, tag="msg")
                    nc.vector.tensor_add(msg[:, 0:nea, :],
                                         hsrc[:, 0:nea, 0:DIM],
                                         esb[:, 0:nea, :])
                    nc.vector.tensor_add(msg[:, nea:nb, :],
                                         hsrc[:, nea:nb, DIM:2 * DIM],
                                         esb[:, nea:nb, :])
                    nc.scalar.activation(msg[:, 0:nb, :], msg[:, 0:nb, :], Act.Relu)

                    # scatter: per tile, accumulate its blocks into PSUM
                    for k, ti in enumerate(tl):
                        aps = psG.tile([DIM, 128], f32, tag="aggps")
                        blocks = ([k * BA + i for i in range(BA)] +
                                  [nea + k * BB + i for i in range(BB)])
                        for j, b in enumerate(blocks):
                            nc.tensor.matmul(aps[:], msg[:, b, :],
                                             ohc[:, b // 4, 128 * (b % 4):
                                                 128 * (b % 4) + 128],
                                             start=(j == 0), stop=(j == NBT - 1))
                        nc.vector.tensor_copy(aggT[:, 128 * ti:128 * (ti + 1)],
                                              aps[:])
                    if ci + DP < nch:
                        prep(l, ci + DP)
                    if l == 0 and 2 <= ci < 10:
                        # generate the layer-invariant pooling one-hots in
                        # layer 0's edge phase and spill to HBM (DVE slack)
                        kk = ci - 2
                        wg, hg = kk // 2, kk % 2
                        HT2 = TILES // 2
                        slab = pohp.tile([128, HT2, 128], bf16, tag="poh",
                                         name="pgen")
                        for tr in range(HT2):
                            ti = hg * HT2 + tr
                            nc.vector.tensor_single_scalar(
                                slab[:, tr, :], iota_sb[:],
                                gid4_sb[:, wg * TILES + ti:
                                        wg * TILES + ti + 1],
                                Alu.is_equal)
                        nc.sync.dma_start(
                            out=poolohs_d[:, wg, hg * HT2:(hg + 1) * HT2, :],
                            in_=slab[:])
                    flush(tl[-1] + 1)

                flush(TILES, final=True)
                if l < NLAYERS - 1:
                    nc.gpsimd.collective_compute(
                        "AllGather", Alu.bypass,
                        replica_groups=[list(range(NCORES))],
                        ins=[bounce[l][:]], outs=[tables[l + 1][:]])
                    for j in range(min(DP, nch)):
                        prep(l + 1, j)

                # ---- pooling for this layer (overlaps the AllGather).
                # One-hots are layer-invariant, generated during layer 0's
                # edge phase and spilled to HBM; every boundary just reloads
                # them, so no is_equal backlog delays the next layer's adds.
                HT2 = TILES // 2
                for w in range(GWIN):
                    pps = psG.tile([128, DIM], f32, tag="aggps", name="pool")
                    for h in range(2):
                        slab = pohp.tile([128, HT2, 128], bf16, tag="poh")
                        nc.sync.dma_start(
                            out=slab[:],
                            in_=poolohs_d[:, w, h * HT2:(h + 1) * HT2, :])
                        for tr in range(HT2):
                            ti = h * HT2 + tr
                            nc.tensor.matmul(pps[:], slab[:, tr, :],
                                             xn_l[:, ti, :],
                                             start=(ti == 0),
                                             stop=(ti == TILES - 1))
                    nc.vector.tensor_copy(pooled[:, w, DIM * l:DIM * (l + 1)],
                                          pps[:])

            # ---------------- pooled AllReduce + head
            nc.sync.dma_start(out=arin_d[:], in_=pooled[:])
            nc.gpsimd.collective_compute(
                "AllReduce", Alu.add, replica_groups=[list(range(NCORES))],
                ins=[arin_d[:]], outs=[arout_d[:]])
            pf = res.tile([128, GWIN, 4 * DIM], bf16, tag="pf")
            nc.sync.dma_start(out=pf[:], in_=arout_d[:])

            # mean + head (pooled layout: [graph-in-window, w, l*64+dim])
            pT = res.tile([128, 2, 512], f32, tag="pT")
            for w in range(GWIN):
                pm = smallp.tile([128, 4 * DIM], f32, tag="pm")
                nc.vector.tensor_scalar_mul(pm[:], pf[:, w, :],
                                            invc_sb[:, w:w + 1])
                for k in range(2):
                    pst = psP.tile([128, 128], f32, tag="pse", name="ph")
                    nc.tensor.transpose(pst[:], pm[:, 128 * k:128 * (k + 1)],
                                        idf_sb[:])
                    nc.vector.tensor_copy(pT[:, k, 128 * w:128 * (w + 1)],
                                          pst[:])
            hps = psA.tile([128, 512], f32, tag="ps1", name="plA")
            for k in range(2):
                nc.tensor.matmul(hps[0:DIM, :], fc1w_sb[:, k, :], pT[:, k, :],
                                 start=(k == 0), stop=(k == 1))
            h1 = res.tile([DIM, 512], f32, tag="h1")
            nc.scalar.activation(h1[:], hps[0:DIM, :], Act.Relu,
                                 bias=fcb_sb["b1"][:])
            hps2 = psB.tile([128, 512], f32, tag="ps2", name="plB")
            nc.tensor.matmul(hps2[0:DIM, :], fc2w_sb[:], h1[:], start=True, stop=True)
            h2 = res.tile([DIM, 512], f32, tag="h2")
            nc.scalar.activation(h2[:], hps2[0:DIM, :], Act.Relu,
                                 bias=fcb_sb["b2"][:])
            hps3 = psA.tile([128, 512], f32, tag="ps1", name="plC")
            nc.tensor.matmul(hps3[0:DIM, :], fc3w_sb[:], h2[:], start=True, stop=True)
            h3 = res.tile([DIM, 512], f32, tag="h3")
            nc.scalar.activation(h3[:], hps3[0:DIM, :], Act.Relu,
                                 bias=fcb_sb["b3"][:])
            hps4 = psB.tile([128, 512], f32, tag="ps2", name="plD")
            nc.tensor.matmul(hps4[0:1, :], fc4w_sb[:], h3[:], start=True, stop=True)
            ho = res.tile([1, G], f32, tag="ho")
            nc.scalar.activation(ho[:], hps4[0:1, :], Act.Identity,
                                 bias=fcb_sb["b4"][:])
            nc.sync.dma_start(out=out_d[:], in_=ho[:])

    nc.compile()
    return nc


# ------------------------------------------------------------------ runner --

_CACHE = {}


def kernel(**inputs):
    from concourse.bass_utils import run_bass_kernel_spmd
    in_maps, struct = _prep(inputs)
    key = (struct["BA"], struct["BB"])
    if key not in _CACHE:
        _CACHE[key] = _build(struct)
    nc = _CACHE[key]
    trace = os.environ.get("BASSGIN_TRACE", "0") == "1"
    res = run_bass_kernel_spmd(nc, in_maps, core_ids=list(range(NCORES)),
                               trace=trace)
    kernel.last_result = res
    out = res.results[0]["out"].reshape(G).astype(np.float32)
    return out
